# revision 32
# baseline (speedup 1.0000x reference)
"""ContraNorm kernel for 8 Trainium2 NeuronCores — fp8 DoubleRow pipeline.

Math (reference):
    norm_x = x / max(||x||_row, eps)
    sim    = (norm_x @ norm_x.T) / tau          # [N, N], tau = 1
    sim[edge_index[0], edge_index[1]] = -inf
    attn   = softmax(sim, axis=1)
    out    = 1.1 * x - 0.1 * (attn @ x)

Sharding: row-parallel.  Core k owns output rows [k*1024, (k+1)*1024).
Each core receives inputs row-rolled so its own rows sit at c-positions
0:1024 — the program is identical on every core (pure SPMD).

Since sim is a cosine similarity in [-1, 1], softmax needs no running
max: exp(sim) is in [e^-1, e].  The -inf edge mask becomes an exact
multiply of exp(sim) by {0, 1}, applied as an integer byte multiply on
the fp8 bit patterns.  The row-sum comes from a ones-column appended to
the V-matmul rhs.

fp8 (e4m3) everywhere on the matmul paths, with DoubleRow perf mode:
  sim:  psum[c,m] = sum_{kt,dp} xt[dp,kt,c] * xt[dp,kt,m]   1 MM / c-chunk
  V:    pv[m,:]  += sum_{kt,cp} et2[cp,kt,m] * xa[cp,kt,:]  4 MM / c-pair
norm_x is pre-scaled by 16 on the host (entries ~N(0,1) in fp8); the
exp activation rescales by 1/256.

The edge mask ships bit-packed (1 MiB/core) and is expanded on-chip to
{0x00, 0xFF} bytes: one fused (x << (7-b)) & 0x80808080 op per bit
puts each mask bit at its byte's MSB, then three in-place
(y >> k) | y passes (k = 1, 2, 4) smear the MSB down the byte — no
cross-byte pollution since the low bits stay zero throughout.  The
mask is then applied as a u32 bitwise AND against the raw fp8 bytes
of exp(sim), 4 bytes per DVE lane-cycle, batched over GRP pairs per
instruction.  All mask work runs on DVE: GpSimd shares its SBUF port
with DVE, so offloading elementwise work there just steals DVE
bandwidth, and integer/bitwise ops are DVE-only anyway.

Per-core inputs (6.07 MiB vs 21 MiB for the dense-bf16-mask variant):
  xt   [128, 2, 8192] fp8   16*norm_x rolled, transposed
  xa   [128, 32, 2, 257] fp8  x rolled (V rhs layout) + ones column
  bits [128, 64, 2, 16] u32  keep-mask bits, b*64+j column mapping
  xo   [1024, 256] f32      own rows for the 1.1*x epilogue term
"""

import numpy as np
import ml_dtypes

N = 8192          # rows of x
D = 256           # features
P = 128           # SBUF partitions
NT = N // P       # 64 c-chunks
R = N // 8        # 1024 rows per core
HALF = 512        # m columns per pass
NPAIR = NT // 2   # 32 c-chunk pairs
SCALE = 0.1
NCORES = 8

GRP = 4           # pairs whose mask multiply is batched into one DVE op

_prog_cache = {}


def _build_program():
    import concourse.bacc as bacc
    import concourse.tile as tile
    from concourse import mybir
    from contextlib import ExitStack

    f32 = mybir.dt.float32
    fp8 = mybir.dt.float8e4
    u32 = mybir.dt.uint32
    u8 = mybir.dt.uint8
    DR = mybir.MatmulPerfMode.DoubleRow
    Exp = mybir.ActivationFunctionType.Exp
    SHR = mybir.AluOpType.logical_shift_right
    SHL = mybir.AluOpType.logical_shift_left
    OR = mybir.AluOpType.bitwise_or
    AND = mybir.AluOpType.bitwise_and
    MUL = mybir.AluOpType.mult
    ADD = mybir.AluOpType.add

    nc = bacc.Bacc("TRN2", target_bir_lowering=False, debug=False)

    xt_h = nc.dram_tensor("xt", [P, 2, N], fp8, kind="ExternalInput")
    xa_h = nc.dram_tensor("xa", [P, NPAIR, 2, D + 1], fp8, kind="ExternalInput")
    bits_h = nc.dram_tensor("bits", [P, 2, NT, 16], u32, kind="ExternalInput")
    xo_h = nc.dram_tensor("xo", [R, D], f32, kind="ExternalInput")
    out_h = nc.dram_tensor("out", [R, D], f32, kind="ExternalOutput")

    xo_d = xo_h.ap().rearrange("(j p) d -> p j d", p=P)    # [128, 8, 256]
    out_d = out_h.ap()

    with ExitStack() as ctx:
        tc = ctx.enter_context(tile.TileContext(nc))

        consts = ctx.enter_context(tc.tile_pool(name="consts", bufs=1))
        maskp = ctx.enter_context(tc.tile_pool(name="maskp", bufs=1))
        work = ctx.enter_context(tc.tile_pool(name="work", bufs=4))
        ps_s = ctx.enter_context(tc.tile_pool(name="ps_s", bufs=2, space="PSUM"))
        ps_v = ctx.enter_context(tc.tile_pool(name="ps_v", bufs=1, space="PSUM"))

        xt = consts.tile([P, 2, N], fp8)
        xa = consts.tile([P, NPAIR, 2, D + 1], fp8)
        bits = consts.tile([P, 2, NT, 16], u32)
        xo = consts.tile([P, R // P, D], f32)

        # bits first (mask expansion is the first compute), then the
        # matmul operands in chunks so compute starts early.
        nc.sync.dma_start(out=bits, in_=bits_h.ap())
        nc.sync.dma_start(out=xt[:, :, 0:R], in_=xt_h.ap()[:, :, 0:R])
        nc.sync.dma_start(out=xt[:, :, R:N], in_=xt_h.ap()[:, :, R:N])
        NXA = 4
        for q in range(NXA):
            sl = slice(q * (NPAIR // NXA), (q + 1) * (NPAIR // NXA))
            nc.sync.dma_start(out=xa[:, sl], in_=xa_h.ap()[:, sl])
        nc.sync.dma_start(out=xo, in_=xo_d)

        # ---- expand all mask bits to {0x00, 0xFF} bytes up front
        # (overlaps with the xt/xa input DMAs; only needs the bits DMA)
        mexp_all = maskp.tile([P, 2, NT, HALF], fp8)
        mexp32 = mexp_all.bitcast(u32)           # [P, 2, NT, 128]
        for b in range(8):
            # bit b -> byte MSB (0x80)
            nc.vector.tensor_scalar(
                out=mexp32[:, :, :, b * 16 : (b + 1) * 16],
                in0=bits,
                scalar1=7 - b,
                scalar2=0x80808080,
                op0=SHL,
                op1=AND,
            )
        # smear the MSB down each byte: 0x80 -> 0xFF
        # (stt immediates lower as f32, which the bitvec verifier rejects;
        # pass the shift amounts as u32 scalar APs instead)
        mflat = mexp32.rearrange("p h t w -> p (h t w)")
        for k in (1, 2, 4):
            kt_ = consts.tile([P, 1], u32, name=f"shk{k}")
            nc.gpsimd.memset(kt_, k)
            nc.vector.scalar_tensor_tensor(
                out=mflat, in0=mflat, scalar=kt_, in1=mflat,
                op0=SHR, op1=OR,
            )

        for h in range(2):
            m0 = h * HALF
            mexp = mexp_all[:, h]                # [P, NT, HALF]
            pv = [
                ps_v.tile([P, D + 1], f32, tag=f"pv{mi}", name=f"pv{mi}")
                for mi in range(4)
            ]
            for gg in range(NPAIR // GRP):
                # et4 holds GRP pairs of exp tiles; masked in one DVE op
                et4 = work.tile([P, GRP, 2, HALF], fp8, tag="et4", bufs=4)
                et4m = work.tile([P, GRP, 2, HALF], fp8, tag="et4m", bufs=4)
                for gi in range(GRP):
                    g = gg * GRP + gi
                    pss = ps_s.tile([P, 2, HALF], f32, tag="pss")
                    for kt in range(2):
                        t = 2 * g + kt
                        nc.tensor.matmul(
                            pss[:, kt, :],
                            xt[:, :, t * P : (t + 1) * P],
                            xt[:, :, m0 : m0 + HALF],
                            start=True,
                            stop=True,
                            perf_mode=DR,
                        )
                    nc.scalar.activation(
                        et4[:, gi].rearrange("p a b -> p (a b)"),
                        pss.rearrange("p a b -> p (a b)"),
                        Exp,
                        scale=1.0 / 256.0,
                    )
                # mask apply: one u32 AND against {0x00,0xFF} bytes per group
                t0 = gg * GRP * 2
                nc.vector.tensor_tensor(
                    out=et4m.rearrange("p a k b -> p (a k b)").bitcast(u32),
                    in0=et4.rearrange("p a k b -> p (a k b)").bitcast(u32),
                    in1=mexp[:, t0 : t0 + 2 * GRP, :].rearrange(
                        "p a b -> p (a b)"
                    ).bitcast(u32),
                    op=AND,
                )
                for gi in range(GRP):
                    g = gg * GRP + gi
                    for mi in range(4):
                        nc.tensor.matmul(
                            pv[mi],
                            et4m[:, gi, :, mi * P : (mi + 1) * P],
                            xa[:, g],
                            start=(g == 0),
                            stop=(g == NPAIR - 1),
                            perf_mode=DR,
                        )
            # ---- epilogue: out = 1.1*x - 0.1 * pv/S ----
            for mi in range(4):
                jj = h * 4 + mi
                sinv = work.tile([P, 1], f32, tag="sinv")
                nc.vector.reciprocal(sinv, pv[mi][:, D : D + 1])
                res = work.tile([P, D], f32, tag="res")
                nc.vector.tensor_scalar(
                    out=res,
                    in0=pv[mi][:, 0:D],
                    scalar1=sinv,
                    scalar2=-SCALE,
                    op0=MUL,
                    op1=MUL,
                )
                nc.vector.scalar_tensor_tensor(
                    out=res,
                    in0=xo[:, jj],
                    scalar=1.0 + SCALE,
                    in1=res,
                    op0=MUL,
                    op1=ADD,
                )
                nc.sync.dma_start(
                    out=out_d[jj * P : (jj + 1) * P, :], in_=res
                )

    nc.compile()
    return nc


def get_program():
    if "prog" not in _prog_cache:
        _prog_cache["prog"] = _build_program()
    return _prog_cache["prog"]


def make_in_maps(x, edge_index):
    fp8 = ml_dtypes.float8_e4m3
    x = np.asarray(x, dtype=np.float32)
    ei = np.asarray(edge_index)
    r = ei[0].astype(np.int64)
    c = ei[1].astype(np.int64)

    norm = np.sqrt((x * x).sum(axis=1, keepdims=True))
    nx16 = np.asarray((x / np.maximum(norm, 1e-12)) * 16.0, dtype=fp8)
    x8 = np.asarray(x, dtype=fp8)

    in_maps = []
    for k in range(NCORES):
        lo = k * R
        nxr = np.roll(nx16, -lo, axis=0)          # [N, D] fp8
        xar = np.roll(x8, -lo, axis=0)            # [N, D] fp8

        # xt[p, kt, c] = nxr[c, kt*128 + p]
        xt = np.ascontiguousarray(
            nxr.T.reshape(2, P, N).transpose(1, 0, 2)
        )
        # xa[p, g, kt, j] = xar[(2g+kt)*128 + p, j], ones at j=256
        xa = np.empty((P, NPAIR, 2, D + 1), dtype=fp8)
        xa[:, :, :, 0:D] = xar.reshape(NPAIR, 2, P, D).transpose(2, 0, 1, 3)
        xa[:, :, :, D] = fp8(1.0)

        # keep-mask, rolled: mask[c_rolled, m_local] = 0 on edges
        sel = (r >= lo) & (r < lo + R)
        m_local = (r[sel] - lo).astype(np.int64)
        c_rolled = (c[sel] - lo) % N
        mask = np.ones((N, R), dtype=np.uint8)
        mask[c_rolled, m_local] = 0
        # column mapping m = h*512 + b*64 + j  ->  byte[c, h, j] bit b
        mm = mask.reshape(N, 2, 8, 64)
        packed = np.packbits(mm, axis=2, bitorder="little")  # [N, 2, 1, 64]
        packed = packed.reshape(N, 2, 64)
        # bits[p, t, h, w] = u32 view of packed[t*128+p, h, 4w:4w+4]
        bits = (
            packed.reshape(NT, P, 2, 64)
            .transpose(1, 2, 0, 3)
            .copy()
            .view("<u4")
        )
        xo = np.ascontiguousarray(x[lo : lo + R])
        in_maps.append({"xt": xt, "xa": xa, "bits": bits, "xo": xo})
    return in_maps


def run(x, edge_index, trace=False):
    from concourse.bass_utils import run_bass_kernel_spmd

    nc = get_program()
    in_maps = make_in_maps(x, edge_index)
    br = run_bass_kernel_spmd(nc, in_maps, list(range(NCORES)), trace=trace)
    out = np.concatenate(
        [br.results[k]["out"] for k in range(NCORES)], axis=0
    ).astype(np.float32)
    return out, br


def kernel(x, edge_index):
    out, _ = run(x, edge_index, trace=False)
    return out


# revision 34
# speedup vs baseline: 1.2607x; 1.2607x over previous
"""ContraNorm kernel for 8 Trainium2 NeuronCores — fp8 DoubleRow pipeline.

Math (reference):
    norm_x = x / max(||x||_row, eps)
    sim    = (norm_x @ norm_x.T) / tau          # [N, N], tau = 1
    sim[edge_index[0], edge_index[1]] = -inf
    attn   = softmax(sim, axis=1)
    out    = 1.1 * x - 0.1 * (attn @ x)

Sharding: row-parallel.  Core k owns output rows [k*1024, (k+1)*1024).
Each core receives inputs row-rolled so its own rows sit at c-positions
0:1024 — the program is identical on every core (pure SPMD).

Since sim is a cosine similarity in [-1, 1], softmax needs no running
max: exp(sim) is in [e^-1, e].  fp8 (e4m3) everywhere on the matmul
paths, with DoubleRow perf mode (2 fp8 k-tiles per instruction):
  sim:  psum[c,m] = sum_{kt,dp} xt[dp,kt,c] * xt[dp,kt,m]   1 MM / c-chunk
  V:    pv[m,:]  += sum_{kt,cp} et[cp,kt,m] * xa[cp,kt,:]   4 MM / c-pair
norm_x is pre-scaled by 4 on the host, so psum = 16*sim and the exp
activation rescales by 1/16.

The -inf edge mask becomes exact zeros in exp(sim), applied two ways
to balance engines (the elementwise pass is the kernel's second wall
after the exp activation itself):
  even chunks (kt0 of each pair): mask bits expanded to {0,1} bytes
      (one fused (x >> b) & 0x01010101 DVE op per bit), then a u8
      integer multiply zeroes masked fp8 bytes of exp(sim) in place.
      Integer/bitwise ops are DVE-only, and GpSimd shares its SBUF
      port with DVE, so all of this stays on DVE.
  odd chunks (kt1): masked inside the PE accumulation: one extra fp8
      matmul per pair adds diag(-240) @ {0,2.0}-mask = -480 to masked
      psum entries; exp((16*sim - 480)/16) = e^(sim-30) underflows to
      exact fp8 zero.  {0, 0x40=2.0} bytes come from the same one-op-
      per-bit extraction, aimed at bit 6.
The host permutes chunks even-first so each format's mask region and
every access pattern stays contiguous.

Per-core inputs (~6.1 MiB vs 21 MiB for a dense-bf16-mask variant —
the end-to-end harness time is dominated by host->device transfer):
  xt    [128, 2, 8192] fp8     4*norm_x rolled, transposed
  xa    [128, 32, 2, 257] fp8  x rolled (V rhs layout) + ones column
  bits  [128, 2, 64, 16] u32   keep-mask bits, even chunks first
  xo    [1024, 256] f32        own rows for the 1.1*x epilogue term
  negid [128, 128] fp8         diag(-240) for the PE mask matmul
"""

import numpy as np
import ml_dtypes

N = 8192          # rows of x
D = 256           # features
P = 128           # SBUF partitions
NT = N // P       # 64 c-chunks
R = N // 8        # 1024 rows per core
HALF = 512        # m columns per pass
NPAIR = NT // 2   # 32 c-chunk pairs
SCALE = 0.1
NCORES = 8
GRP = 4           # pairs per mask-multiply batch

# chunk order as seen by the mask bits: evens (u8-masked) then odds
# (PE-masked)
CHUNK_ORDER = list(range(0, NT, 2)) + list(range(1, NT, 2))

_prog_cache = {}


def _build_program():
    import concourse.bacc as bacc
    import concourse.tile as tile
    from concourse import mybir
    from contextlib import ExitStack

    f32 = mybir.dt.float32
    fp8 = mybir.dt.float8e4
    u32 = mybir.dt.uint32
    u8 = mybir.dt.uint8
    DR = mybir.MatmulPerfMode.DoubleRow
    Exp = mybir.ActivationFunctionType.Exp
    SHR = mybir.AluOpType.logical_shift_right
    SHL = mybir.AluOpType.logical_shift_left
    AND = mybir.AluOpType.bitwise_and
    MUL = mybir.AluOpType.mult
    ADD = mybir.AluOpType.add

    nc = bacc.Bacc("TRN2", target_bir_lowering=False, debug=False)

    xt_h = nc.dram_tensor("xt", [P, 2, N], fp8, kind="ExternalInput")
    xa_h = nc.dram_tensor("xa", [P, NPAIR, 2, D + 1], fp8, kind="ExternalInput")
    bits_h = nc.dram_tensor("bits", [P, 2, NT, 16], u32, kind="ExternalInput")
    xo_h = nc.dram_tensor("xo", [R, D], f32, kind="ExternalInput")
    negid_h = nc.dram_tensor("negid", [P, P], fp8, kind="ExternalInput")
    out_h = nc.dram_tensor("out", [R, D], f32, kind="ExternalOutput")

    xo_d = xo_h.ap().rearrange("(j p) d -> p j d", p=P)    # [128, 8, 256]
    out_d = out_h.ap()

    with ExitStack() as ctx:
        tc = ctx.enter_context(tile.TileContext(nc))

        consts = ctx.enter_context(tc.tile_pool(name="consts", bufs=1))
        maskp = ctx.enter_context(tc.tile_pool(name="maskp", bufs=1))
        work = ctx.enter_context(tc.tile_pool(name="work", bufs=4))
        ps_s = ctx.enter_context(tc.tile_pool(name="ps_s", bufs=2, space="PSUM"))
        ps_v = ctx.enter_context(tc.tile_pool(name="ps_v", bufs=1, space="PSUM"))

        xt = consts.tile([P, 2, N], fp8)
        xa = consts.tile([P, NPAIR, 2, D + 1], fp8)
        bits = consts.tile([P, 2, NT, 16], u32)
        xo = consts.tile([P, R // P, D], f32)
        negid = consts.tile([P, P], fp8)

        # bits first (mask expansion is the first compute), then the
        # matmul operands in chunks so compute starts early.
        nc.sync.dma_start(out=bits, in_=bits_h.ap())
        nc.sync.dma_start(out=negid, in_=negid_h.ap())
        nc.sync.dma_start(out=xt[:, :, 0:R], in_=xt_h.ap()[:, :, 0:R])
        nc.sync.dma_start(out=xt[:, :, R:N], in_=xt_h.ap()[:, :, R:N])
        NXA = 4
        for q in range(NXA):
            sl = slice(q * (NPAIR // NXA), (q + 1) * (NPAIR // NXA))
            nc.sync.dma_start(out=xa[:, sl], in_=xa_h.ap()[:, sl])
        nc.sync.dma_start(out=xo, in_=xo_d)

        # ---- expand all mask bits up front (overlaps the xt/xa DMAs)
        # layout [P, 2(h), NT, HALF]: chunks 0:32 = even chunks ({0,1}
        # bytes for the u8 multiply), 32:64 = odd chunks ({0,0x40}=2.0
        # for the PE mask matmul)
        mexp_all = maskp.tile([P, 2, NT, HALF], fp8)
        mexp32 = mexp_all.bitcast(u32)           # [P, 2, NT, 128]
        H2 = NT // 2
        for b in range(8):
            nc.vector.tensor_scalar(
                out=mexp32[:, :, 0:H2, b * 16 : (b + 1) * 16],
                in0=bits[:, :, 0:H2, :],
                scalar1=b,
                scalar2=0x01010101,
                op0=SHR,
                op1=AND,
            )
            nc.vector.tensor_scalar(
                out=mexp32[:, :, H2:NT, b * 16 : (b + 1) * 16],
                in0=bits[:, :, H2:NT, :],
                scalar1=(6 - b) if b < 7 else 1,
                scalar2=0x40404040,
                op0=SHL if b < 7 else SHR,
                op1=AND,
            )

        for h in range(2):
            m0 = h * HALF
            mexp = mexp_all[:, h]                # [P, NT, HALF]
            pv = [
                ps_v.tile([P, D + 1], f32, tag=f"pv{mi}", name=f"pv{mi}")
                for mi in range(4)
            ]
            for gg in range(NPAIR // GRP):
                et4 = work.tile([P, GRP, 2, HALF], fp8, tag="et4", bufs=4)
                for gi in range(GRP):
                    g = gg * GRP + gi
                    pss = ps_s.tile([P, 2, HALF], f32, tag="pss")
                    # kt0 (even chunk): plain sim matmul
                    nc.tensor.matmul(
                        pss[:, 0, :],
                        xt[:, :, (2 * g) * P : (2 * g + 1) * P],
                        xt[:, :, m0 : m0 + HALF],
                        start=True,
                        stop=True,
                        perf_mode=DR,
                    )
                    # kt1 (odd chunk): sim matmul + PE mask bias
                    nc.tensor.matmul(
                        pss[:, 1, :],
                        xt[:, :, (2 * g + 1) * P : (2 * g + 2) * P],
                        xt[:, :, m0 : m0 + HALF],
                        start=True,
                        stop=False,
                        perf_mode=DR,
                    )
                    nc.tensor.matmul(
                        pss[:, 1, :],
                        negid,
                        mexp[:, H2 + g, :],
                        start=False,
                        stop=True,
                    )
                    nc.scalar.activation(
                        et4[:, gi].rearrange("p a b -> p (a b)"),
                        pss.rearrange("p a b -> p (a b)"),
                        Exp,
                        scale=1.0 / 16.0,
                    )
                # mask the even chunks: u8 multiply by {0,1}, in place
                nc.vector.tensor_tensor(
                    out=et4[:, :, 0, :].bitcast(u8),
                    in0=et4[:, :, 0, :].bitcast(u8),
                    in1=mexp[:, GRP * gg : GRP * (gg + 1), :].rearrange(
                        "p a b -> p (a b)"
                    ).bitcast(u8),
                    op=MUL,
                )
                for gi in range(GRP):
                    g = gg * GRP + gi
                    for mi in range(4):
                        nc.tensor.matmul(
                            pv[mi],
                            et4[:, gi, :, mi * P : (mi + 1) * P],
                            xa[:, g],
                            start=(g == 0),
                            stop=(g == NPAIR - 1),
                            perf_mode=DR,
                        )
            # ---- epilogue: out = 1.1*x - 0.1 * pv/S ----
            for mi in range(4):
                jj = h * 4 + mi
                sinv = work.tile([P, 1], f32, tag="sinv")
                nc.vector.reciprocal(sinv, pv[mi][:, D : D + 1])
                res = work.tile([P, D], f32, tag="res")
                nc.vector.tensor_scalar(
                    out=res,
                    in0=pv[mi][:, 0:D],
                    scalar1=sinv,
                    scalar2=-SCALE,
                    op0=MUL,
                    op1=MUL,
                )
                nc.vector.scalar_tensor_tensor(
                    out=res,
                    in0=xo[:, jj],
                    scalar=1.0 + SCALE,
                    in1=res,
                    op0=MUL,
                    op1=ADD,
                )
                nc.sync.dma_start(
                    out=out_d[jj * P : (jj + 1) * P, :], in_=res
                )

    nc.compile()
    return nc


def get_program():
    if "prog" not in _prog_cache:
        _prog_cache["prog"] = _build_program()
    return _prog_cache["prog"]


def make_in_maps(x, edge_index):
    fp8 = ml_dtypes.float8_e4m3
    x = np.asarray(x, dtype=np.float32)
    ei = np.asarray(edge_index)
    r = ei[0].astype(np.int64)
    c = ei[1].astype(np.int64)

    norm = np.sqrt((x * x).sum(axis=1, keepdims=True))
    nx4 = np.asarray((x / np.maximum(norm, 1e-12)) * 4.0, dtype=fp8)
    x8 = np.asarray(x, dtype=fp8)

    negid = np.zeros((P, P), dtype=fp8)
    np.fill_diagonal(negid, fp8(-240.0))

    in_maps = []
    for k in range(NCORES):
        lo = k * R
        nxr = np.roll(nx4, -lo, axis=0)           # [N, D] fp8
        xar = np.roll(x8, -lo, axis=0)            # [N, D] fp8

        # xt[p, kt, c] = nxr[c, kt*128 + p]
        xt = np.ascontiguousarray(
            nxr.T.reshape(2, P, N).transpose(1, 0, 2)
        )
        # xa[p, g, kt, j] = xar[(2g+kt)*128 + p, j], ones at j=256
        xa = np.empty((P, NPAIR, 2, D + 1), dtype=fp8)
        xa[:, :, :, 0:D] = xar.reshape(NPAIR, 2, P, D).transpose(2, 0, 1, 3)
        xa[:, :, :, D] = fp8(1.0)

        # keep-mask, rolled: mask[c_rolled, m_local] = 0 on edges
        sel = (r >= lo) & (r < lo + R)
        m_local = (r[sel] - lo).astype(np.int64)
        c_rolled = (c[sel] - lo) % N
        mask = np.ones((N, R), dtype=np.uint8)
        mask[c_rolled, m_local] = 0
        # column mapping m = h*512 + b*64 + j  ->  byte[c, h, j] bit b
        mm = mask.reshape(N, 2, 8, 64)
        packed = np.packbits(mm, axis=2, bitorder="little")  # [N, 2, 1, 64]
        packed = packed.reshape(N, 2, 64)
        # bits[p, h, t, w]: chunk order = evens then odds.  The odd
        # (PE-masked) region is inverted: its extraction must yield
        # 2.0 at MASKED positions (the -480 psum bias), not at kept.
        bits = (
            packed.reshape(NT, P, 2, 64)[CHUNK_ORDER]
            .transpose(1, 2, 0, 3)
            .copy()
        )
        bits[:, :, NT // 2 :, :] = ~bits[:, :, NT // 2 :, :]
        bits = bits.view("<u4")
        xo = np.ascontiguousarray(x[lo : lo + R])
        in_maps.append(
            {"xt": xt, "xa": xa, "bits": bits, "xo": xo, "negid": negid}
        )
    return in_maps


def run(x, edge_index, trace=False):
    from concourse.bass_utils import run_bass_kernel_spmd

    nc = get_program()
    in_maps = make_in_maps(x, edge_index)
    br = run_bass_kernel_spmd(nc, in_maps, list(range(NCORES)), trace=trace)
    out = np.concatenate(
        [br.results[k]["out"] for k in range(NCORES)], axis=0
    ).astype(np.float32)
    return out, br


def kernel(x, edge_index):
    out, _ = run(x, edge_index, trace=False)
    return out
